# revision 9
# baseline (speedup 1.0000x reference)
"""Chamfer loss kernel for Trainium2 (8 NeuronCores, Bass/Tile).

Problem: x, y of shape [8192, 128] fp32.
  dist[i,j] = max(||x_i||^2 + ||y_j||^2 - 2 x_i.y_j, 0)
  loss = (sum_j min_i dist[i,j] + sum_i min_j dist[i,j]) / 8192

Sharding: x rows are split across the 8 cores (1024 rows each); every core
holds all of y. Each core computes its [1024, 8192] distance tile via PE
matmuls (K = 128 features on partitions):

  PSUM tile = (-2 x_chunk^T)^T @ y^T            (K=128 fp16 matmul)
            + [ones; x2_chunk]^T @ [y2; ones]   (K=2 rank-2 bias fold)
  => PSUM[i, j] = d_sh = dist - S  (shifted squared distance)

Exp-space reductions (softmin rows / exact monotone cols):
  ScalarE drains PSUM with  E = exp((C - d_sh)/T)  (bf16), and its free
  accum_out gives per-partition rowsum partials  sum_j E  -- the entire
  per-x row reduction costs nothing beyond the drain. VectorE only
  accumulates per-y column maxima  colacc = max(colacc, E)  elementwise
  across the 8 x-chunks (exp is monotone, so max E <-> min dist exactly).
  A DMA transpose + bf16 max-tree reduces colacc across partitions.

Each core outputs parts[128, 64+32] = [col max-reduced | rowsum partials];
the host does the tiny final math:  min_dist = S + C - T*log(.)  plus the
cross-core max/sum, clamp and mean. Softmin bias at T=1.25 is ~7e-4 rel.
"""

import os
import sys

import numpy as np

sys.path.insert(0, "/opt/trn_rl_repo")
os.environ.setdefault("MYCRO_LOCAL_CACHE", "1")

import concourse.bass as bass
import concourse.bacc as bacc
import concourse.mybir as mybir
import concourse.tile as tile

FP16 = mybir.dt.float16
BF16 = mybir.dt.bfloat16
FP32 = mybir.dt.float32
FP8 = mybir.dt.float8e4
AF = mybir.ActivationFunctionType
ALU = mybir.AluOpType
DR = mybir.MatmulPerfMode.DoubleRow

NPTS = 8192          # points in x and in y
DIM = 128            # feature dim = matmul contraction K
NCORES = 8
LOCAL = NPTS // NCORES   # 1024 x-rows per core
P = 128              # partitions
CHUNKS = LOCAL // P      # 8 chunks of 128 x-rows per core
JT = 512             # j-tile width (one PSUM bank of fp32)
GROUP = 4            # j-tiles per PSUM pool buffer / ACT drain
NGRP = NPTS // (JT * GROUP)  # 4 drain groups per chunk
NBLK = NPTS // P     # 64 column blocks of 128 y-points
NRS = CHUNKS * NGRP  # 32 rowsum partial slots

TSOFT = 1.25         # softmin temperature (baked into the NEFF)

EBUFS = int(os.environ.get("K_EBUFS", "3"))
COLACC_GRAIN = int(os.environ.get("K_COLACC_GRAIN", "2048"))


def _build_module(stage: str = "host"):
    """stage: 'host' (per-core partials, host combine) | 'main' (chunk loop
    only) | 'pe' (matmuls only) | 'pedrain' (matmuls + exp drain).
    A digit suffix repeats the body (e.g. 'host17')."""
    nrep = 1
    for base in ("host", "pedrain", "pe", "main"):
        if stage.startswith(base):
            suf = stage[len(base):]
            if suf.isdigit():
                nrep = int(suf)
            stage = base
            break
    nc = bacc.Bacc(
        "TRN2",
        target_bir_lowering=False,
        debug=False,
        num_devices=NCORES,
    )

    # DoubleRow fp8 layouts: feature f -> (k_i = f % 64, k_o = f // 64)
    xT2 = nc.dram_tensor("xT2", [P // 2, 2, LOCAL], FP8, kind="ExternalInput")
    yT = nc.dram_tensor("yT", [P // 2, 2, NPTS], FP8, kind="ExternalInput")
    fold_lhsT = nc.dram_tensor("fold_lhsT", [2, LOCAL], FP16, kind="ExternalInput")
    fold_rhs = nc.dram_tensor("fold_rhs", [2, NPTS], FP16, kind="ExternalInput")
    actbias = nc.dram_tensor("actbias", [P, 1], FP32, kind="ExternalInput")
    # cols 0..63: per-y col max of E (cross-partition reduced); cols 64..95:
    # rowsum partials per (chunk, group)
    parts = nc.dram_tensor("parts", [P, NBLK + NRS], FP32, kind="ExternalOutput")

    with tile.TileContext(nc) as tc:
        with (
            tc.tile_pool(name="const", bufs=1) as cpool,
            tc.tile_pool(name="big", bufs=1) as bigpool,
            tc.tile_pool(name="epool", bufs=EBUFS) as epool,
            tc.tile_pool(name="scratch", bufs=1) as spool,
        ):
            sb_xT2 = cpool.tile([P // 2, 2, LOCAL], FP8, tag="xT2")
            sb_yT = cpool.tile([P // 2, 2, NPTS], FP8, tag="yT")
            sb_flhs = cpool.tile([2, LOCAL], FP16, tag="flhs")
            sb_frhs = cpool.tile([2, NPTS], FP16, tag="frhs")
            sb_bias = cpool.tile([P, 1], FP32, tag="abias")

            nc.sync.dma_start(sb_yT[:], yT[:])
            nc.sync.dma_start(sb_xT2[:], xT2[:])
            nc.sync.dma_start(sb_flhs[:], fold_lhsT[:])
            nc.sync.dma_start(sb_frhs[:], fold_rhs[:])
            nc.sync.dma_start(sb_bias[:], actbias[:])

            # two column-max accumulators: A over chunks 0-3, B over 4-7.
            # A finalizes (transpose + partition max-tree) while B's chunks
            # still run, hiding half the tail.
            colaccA = bigpool.tile([P, NPTS], BF16, tag="colaccA")
            colaccB = bigpool.tile([P, NPTS], BF16, tag="colaccB")
            colaccT = bigpool.tile([P, NBLK, P], BF16, tag="colaccT")
            ctree = bigpool.tile([P, NBLK, P // 2], BF16, tag="ctree")
            cmaxA = spool.tile([P, NBLK], BF16, tag="cmaxA")
            cmaxB = spool.tile([P, NBLK], BF16, tag="cmaxB")
            rowsums = spool.tile([P, NRS], FP32, tag="rowsums")

            def _ctree_reduce(dst_small, b0=0, b1=NBLK):
                """Halving max-tree over colaccT's last axis for blocks
                [b0:b1]; result copied to dst_small[:, b0:b1]."""
                w = P // 2
                nc.vector.tensor_tensor(
                    ctree[:, b0:b1, 0:w], colaccT[:, b0:b1, 0:w],
                    colaccT[:, b0:b1, w:P], op=ALU.max,
                )
                while w > 1:
                    h2 = w // 2
                    nc.vector.tensor_tensor(
                        ctree[:, b0:b1, 0:h2], ctree[:, b0:b1, 0:h2],
                        ctree[:, b0:b1, h2:w], op=ALU.max,
                    )
                    w = h2
                nc.vector.tensor_copy(dst_small[:, b0:b1], ctree[:, b0:b1, 0])

            with tc.tile_pool(name="psum", bufs=2, space="PSUM") as psum_pool:
              for rep in range(nrep):
                for c in range(CHUNKS):
                    acc_direct = colaccA if c == 0 else (
                        colaccB if c == CHUNKS // 2 else None)
                    if acc_direct is not None and stage not in ("pe",):
                        # first chunk of an accumulator: ACT drains straight
                        # into it -- no separate DVE max needed
                        e_c = acc_direct
                    else:
                        e_c = epool.tile([P, NPTS], BF16, tag="E")
                    csl = bass.ts(c, P)
                    for g in range(NGRP):
                        pt = psum_pool.tile([P, GROUP * JT], FP32, tag="D")
                        # main matmuls of the group share one lhsT load;
                        # the K=2 bias folds share another.
                        for t in range(GROUP):
                            j0 = (g * GROUP + t) * JT
                            nc.tensor.matmul(
                                pt[:, bass.ts(t, JT)],
                                lhsT=sb_xT2[:, :, csl],
                                rhs=sb_yT[:, :, j0 : j0 + JT],
                                start=True,
                                stop=False,
                                perf_mode=DR,
                            )
                        for t in range(GROUP):
                            j0 = (g * GROUP + t) * JT
                            nc.tensor.matmul(
                                pt[:, bass.ts(t, JT)],
                                lhsT=sb_flhs[:, csl],
                                rhs=sb_frhs[:, j0 : j0 + JT],
                                start=False,
                                stop=True,
                            )
                        gsl = bass.ts(g, GROUP * JT)
                        if stage == "pe":
                            # keep a consumer so matmuls aren't dead: tiny
                            # copy of one column per group
                            nc.scalar.activation(
                                e_c[:, g : g + 1], pt[:, 0:1], AF.Copy
                            )
                        else:
                            # E = exp((C - d_sh)/T), rowsum partial for free
                            nc.scalar.activation(
                                e_c[:, gsl], pt[:], AF.Exp,
                                bias=sb_bias[:, 0:1],
                                scale=-1.0 / TSOFT,
                                accum_out=rowsums[:, c * NGRP + g
                                                  : c * NGRP + g + 1],
                            )

                    if stage in ("pe", "pedrain"):
                        # tiny reader keeps each chunk's work live
                        nc.vector.tensor_copy(
                            cmaxA[0:1, c : c + 1], e_c[0:1, 0:1]
                        )
                        continue
                    # per-y col maxima accumulated elementwise across chunks
                    colacc = colaccA if c < CHUNKS // 2 else colaccB
                    if acc_direct is None:
                        for j0 in range(0, NPTS, COLACC_GRAIN):
                            sl = slice(j0, j0 + COLACC_GRAIN)
                            nc.vector.tensor_tensor(
                                colacc[:, sl], e_c[:, sl], colacc[:, sl],
                                op=ALU.max,
                            )
                    if c == CHUNKS // 2 - 1 and stage == "host":
                        # finalize A while B's chunks still run
                        nc.sync.dma_start_transpose(colaccT[:], colaccA[:])
                        _ctree_reduce(cmaxA)

                if stage == "host":
                    # finalize B (transpose in two halves so the max-tree on
                    # half 0 overlaps the DMA of half 1), then output
                    half = NPTS // 2
                    hb = NBLK // 2
                    nc.sync.dma_start_transpose(
                        colaccT[:, 0:hb, :], colaccB[:, 0:half])
                    nc.sync.dma_start_transpose(
                        colaccT[:, hb:NBLK, :], colaccB[:, half:NPTS])
                    _ctree_reduce(cmaxB, 0, hb)
                    _ctree_reduce(cmaxB, hb, NBLK)
                    cmax = spool.tile([P, NBLK], FP32, tag="cmax")
                    nc.vector.tensor_tensor(
                        cmax[:], cmaxA[:], cmaxB[:], op=ALU.max)
                    nc.sync.dma_start(parts[:, 0:NBLK], cmax[:])
                    nc.sync.dma_start(parts[:, NBLK:], rowsums[:])
                elif stage == "main":
                    lres0 = spool.tile([1, 2], FP32, tag="lres0")
                    nc.vector.tensor_copy(lres0[:, 0:1], rowsums[0:1, 0:1])
                    nc.vector.tensor_copy(lres0[:, 1:2], colaccB[0:1, 0:1])
                    nc.sync.dma_start(parts[0:1, 0:2], lres0[:])
                else:
                    lres0 = spool.tile([1, 1], FP32, tag="lres0")
                    nc.vector.tensor_copy(lres0[:], cmaxA[0:1, 0:1])
                    nc.sync.dma_start(parts[0:1, 0:1], lres0[:])

    nc.compile()
    return nc


_NC_CACHE: dict = {}


def _get_module(stage: str = "host"):
    if stage not in _NC_CACHE:
        _NC_CACHE[stage] = _build_module(stage)
    return _NC_CACHE[stage]


_RUNNER_CACHE: dict = {}


def _get_runner(stage: str = "host", donate: bool = True):
    """Build (once) a jitted SPMD callable over the 8 cores."""
    key = (stage, donate)
    if key in _RUNNER_CACHE:
        return _RUNNER_CACHE[key]

    import jax
    from jax.sharding import Mesh, PartitionSpec
    from jax.experimental.shard_map import shard_map
    import concourse.mybir as _mybir
    from concourse import bass2jax

    nc = _get_module(stage)
    bass2jax.install_neuronx_cc_hook()

    partition_name = (
        nc.partition_id_tensor.name if nc.partition_id_tensor else None
    )
    in_names: list[str] = []
    out_names: list[str] = []
    out_avals: list[jax.core.ShapedArray] = []
    zero_outs: list[np.ndarray] = []
    for alloc in nc.m.functions[0].allocations:
        if not isinstance(alloc, _mybir.MemoryLocationSet):
            continue
        name = alloc.memorylocations[0].name
        if alloc.kind == "ExternalInput":
            if name != partition_name:
                in_names.append(name)
        elif alloc.kind == "ExternalOutput":
            out_names.append(name)
            shape = tuple(alloc.tensor_shape)
            dtype = _mybir.dt.np(alloc.dtype)
            out_avals.append(jax.core.ShapedArray(shape, dtype))
            zero_outs.append(np.zeros(shape, dtype))
    n_params = len(in_names)
    n_outs = len(out_avals)
    all_names = in_names + out_names
    if partition_name is not None:
        all_names = all_names + [partition_name]

    def _body(*args):
        operands = list(args)
        if partition_name is not None:
            operands.append(bass2jax.partition_id_tensor())
        outs = bass2jax._bass_exec_p.bind(
            *operands,
            out_avals=tuple(out_avals),
            in_names=tuple(all_names),
            out_names=tuple(out_names),
            lowering_input_output_aliases=(),
            sim_require_finite=True,
            sim_require_nnan=True,
            nc=nc,
        )
        return tuple(outs)

    devices = jax.devices()[:NCORES]
    mesh = Mesh(np.asarray(devices), ("core",))
    in_specs = (PartitionSpec("core"),) * (n_params + n_outs)
    out_specs = (PartitionSpec("core"),) * n_outs
    jit_kw = (
        dict(donate_argnums=tuple(range(n_params, n_params + n_outs)))
        if donate
        else {}
    )
    sharded = jax.jit(
        shard_map(_body, mesh=mesh, in_specs=in_specs, out_specs=out_specs,
                  check_rep=False),
        keep_unused=True,
        **jit_kw,
    )
    _RUNNER_CACHE[key] = (sharded, in_names, out_names, out_avals, zero_outs,
                          mesh)
    return _RUNNER_CACHE[key]


def _run(in_maps, stage="host"):
    sharded, in_names, out_names, out_avals, zero_outs, _ = _get_runner(stage)
    concat_in = [
        np.concatenate([np.asarray(in_maps[c][n]) for c in range(NCORES)], axis=0)
        for n in in_names
    ]
    concat_zeros = [
        np.zeros((NCORES * z.shape[0], *z.shape[1:]), z.dtype) for z in zero_outs
    ]
    out_arrs = sharded(*concat_in, *concat_zeros)
    return [
        {
            n: np.asarray(out_arrs[i]).reshape(NCORES, *out_avals[i].shape)[c]
            for i, n in enumerate(out_names)
        }
        for c in range(NCORES)
    ]


def _prep_inputs(x: np.ndarray, y: np.ndarray):
    return _prep_inputs_s(x, y)[0]


def _prep_inputs_s(x: np.ndarray, y: np.ndarray):
    x = np.asarray(x, np.float32)
    y = np.asarray(y, np.float32)
    x2 = np.sum(x.astype(np.float64) ** 2, axis=1)
    y2 = np.sum(y.astype(np.float64) ** 2, axis=1)
    s = float(x2.min() + y2.min())
    x2s = (x2 - x2.min()).astype(np.float32)
    y2s = (y2 - y2.min()).astype(np.float32)

    import ml_dtypes

    f8 = np.dtype(ml_dtypes.float8_e4m3)
    # DoubleRow interleave: [features, n] -> [64, 2, n]
    yT = np.ascontiguousarray(
        y.T.reshape(2, P // 2, NPTS).transpose(1, 0, 2)
    ).astype(f8)
    fold_rhs = np.empty((2, NPTS), np.float16)
    fold_rhs[0] = y2s.astype(np.float16)
    fold_rhs[1] = 1.0

    # softmin reference point: median row min (shifted space) over a small
    # row sample; exact value only affects dynamic range, not correctness
    samp = x[:: NPTS // 16][:16]
    samp2 = x2s[:: NPTS // 16][:16]
    d_samp = samp2[:, None] + y2s[None, :] - 2.0 * (samp @ y.T)
    c_sh = float(np.median(d_samp.min(axis=1)))
    actbias = np.full((P, 1), c_sh / TSOFT, np.float32)

    in_maps = []
    for c in range(NCORES):
        sl = slice(c * LOCAL, (c + 1) * LOCAL)
        xT2 = np.ascontiguousarray(
            (-2.0 * x[sl]).T.reshape(2, P // 2, LOCAL).transpose(1, 0, 2)
        ).astype(f8)
        fold_lhsT = np.empty((2, LOCAL), np.float16)
        fold_lhsT[0] = 1.0
        fold_lhsT[1] = x2s[sl].astype(np.float16)
        in_maps.append(
            {
                "xT2": xT2,
                "yT": yT,
                "fold_lhsT": fold_lhsT,
                "fold_rhs": fold_rhs,
                "actbias": actbias,
            }
        )
    return in_maps, s, c_sh


def kernel(x: np.ndarray, y: np.ndarray, **_ignored):
    x = np.asarray(x, np.float32)
    y = np.asarray(y, np.float32)
    in_maps, s, c_sh = _prep_inputs_s(x, y)
    results = _run(in_maps, stage="host")
    parts = np.stack([results[c]["parts"] for c in range(NCORES)])  # [8,128,96]
    colmax = parts[:, :, 0:NBLK].astype(np.float64).max(axis=0)  # [128, 64]
    rowsum = parts[:, :, NBLK:].astype(np.float64)
    rowsum = rowsum.reshape(NCORES, P, CHUNKS, NGRP).sum(axis=3)  # [8,128,8]
    colmin = s + c_sh - TSOFT * np.log(colmax)
    rowmin = s + c_sh - TSOFT * np.log(rowsum)
    loss = (
        np.maximum(colmin, 0.0).sum() + np.maximum(rowmin, 0.0).sum()
    ) / NPTS
    return np.float32(loss)


# revision 14
# speedup vs baseline: 1.7333x; 1.7333x over previous
"""Chamfer loss kernel for Trainium2 (8 NeuronCores, Bass/Tile).

Problem: x, y of shape [8192, 128] fp32.
  dist[i,j] = max(||x_i||^2 + ||y_j||^2 - 2 x_i.y_j, 0)
  loss = (sum_j min_i dist[i,j] + sum_i min_j dist[i,j]) / 8192

Sharding: x rows are split across the 8 cores (1024 rows each); every core
holds all of y. Each core computes its [1024, 8192] distance tile via PE
matmuls (K = 128 features on partitions):

  PSUM tile = (-2 x_chunk^T)^T @ y^T            (K=128 fp16 matmul)
            + [ones; x2_chunk]^T @ [y2; ones]   (K=2 rank-2 bias fold)
  => PSUM[i, j] = d_sh = dist - S  (shifted squared distance)

Exp-space reductions (softmin rows / exact monotone cols):
  ScalarE drains PSUM with  E = exp((C - d_sh)/T)  (bf16), and its free
  accum_out gives per-partition rowsum partials  sum_j E  -- the entire
  per-x row reduction costs nothing beyond the drain. VectorE only
  accumulates per-y column maxima  colacc = max(colacc, E)  elementwise
  across the 8 x-chunks (exp is monotone, so max E <-> min dist exactly).
  A DMA transpose + bf16 max-tree reduces colacc across partitions.

Each core outputs parts[128, 64+32] = [col max-reduced | rowsum partials];
the host does the tiny final math:  min_dist = S + C - T*log(.)  plus the
cross-core max/sum, clamp and mean. Softmin bias at T=1.25 is ~7e-4 rel.
"""

import os
import sys

import numpy as np

sys.path.insert(0, "/opt/trn_rl_repo")
os.environ.setdefault("MYCRO_LOCAL_CACHE", "1")

import concourse.bass as bass
import concourse.bacc as bacc
import concourse.mybir as mybir
import concourse.tile as tile

FP16 = mybir.dt.float16
BF16 = mybir.dt.bfloat16
FP32 = mybir.dt.float32
FP8 = mybir.dt.float8e4
AF = mybir.ActivationFunctionType
ALU = mybir.AluOpType
DR = mybir.MatmulPerfMode.DoubleRow

NPTS = 8192          # points in x and in y
DIM = 128            # feature dim = matmul contraction K
NCORES = 8
LOCAL = NPTS // NCORES   # 1024 x-rows per core
P = 128              # partitions
CHUNKS = LOCAL // P      # 8 chunks of 128 x-rows per core
JT = 512             # j-tile width (one PSUM bank of fp32)
GROUP = 4            # j-tiles per PSUM pool buffer / ACT drain
NGRP = NPTS // (JT * GROUP)  # 4 drain groups per chunk
NBLK = NPTS // P     # 64 column blocks of 128 y-points
NRS = CHUNKS * NGRP  # 32 rowsum partial slots

TSOFT = 1.25         # softmin temperature (baked into the NEFF)

EBUFS = int(os.environ.get("K_EBUFS", "3"))
COLACC_GRAIN = int(os.environ.get("K_COLACC_GRAIN", "2048"))


def _build_module(stage: str = "host"):
    """stage: 'host' (per-core partials, host combine) | 'main' (chunk loop
    only) | 'pe' (matmuls only) | 'pedrain' (matmuls + exp drain).
    A digit suffix repeats the body (e.g. 'host17')."""
    nrep = 1
    for base in ("host", "pedrain", "pe", "main"):
        if stage.startswith(base):
            suf = stage[len(base):]
            if suf.isdigit():
                nrep = int(suf)
            stage = base
            break
    nc = bacc.Bacc(
        "TRN2",
        target_bir_lowering=False,
        debug=False,
        num_devices=NCORES,
    )

    # DoubleRow fp8 layouts: feature f -> (k_i = f % 64, k_o = f // 64)
    xT2 = nc.dram_tensor("xT2", [P // 2, 2, LOCAL], FP8, kind="ExternalInput")
    yT = nc.dram_tensor("yT", [P // 2, 2, NPTS], FP8, kind="ExternalInput")
    # K=6 fp8 fold: [1,1,1,x2hi,x2mid,x2lo] x [y2hi,y2mid,y2lo,1,1,1]
    fold_lhsT = nc.dram_tensor("fold_lhsT", [3, 2, LOCAL], FP8, kind="ExternalInput")
    fold_rhs = nc.dram_tensor("fold_rhs", [3, 2, NPTS], FP8, kind="ExternalInput")
    actbias = nc.dram_tensor("actbias", [P, 1], FP32, kind="ExternalInput")
    # cols 0..63: per-y col max of E (cross-partition reduced); cols 64..95:
    # rowsum partials per (chunk, group)
    parts = nc.dram_tensor("parts", [P, NBLK + NRS], FP32, kind="ExternalOutput")

    with tile.TileContext(nc) as tc:
        with (
            tc.tile_pool(name="const", bufs=1) as cpool,
            tc.tile_pool(name="big", bufs=1) as bigpool,
            tc.tile_pool(name="epool", bufs=EBUFS) as epool,
            tc.tile_pool(name="scratch", bufs=1) as spool,
        ):
            sb_xT2 = cpool.tile([P // 2, 2, LOCAL], FP8, tag="xT2")
            sb_yT = cpool.tile([P // 2, 2, NPTS], FP8, tag="yT")
            sb_flhs = cpool.tile([3, 2, LOCAL], FP8, tag="flhs")
            sb_frhs = cpool.tile([3, 2, NPTS], FP8, tag="frhs")
            sb_bias = cpool.tile([P, 1], FP32, tag="abias")

            nc.sync.dma_start(sb_yT[:], yT[:])
            nc.sync.dma_start(sb_xT2[:], xT2[:])
            nc.sync.dma_start(sb_flhs[:], fold_lhsT[:])
            nc.sync.dma_start(sb_frhs[:], fold_rhs[:])
            nc.sync.dma_start(sb_bias[:], actbias[:])

            # two column-max accumulators: A over chunks 0-3, B over 4-7.
            # A finalizes (transpose + partition max-tree) while B's chunks
            # still run, hiding half the tail.
            colaccA = bigpool.tile([P, NPTS], BF16, tag="colaccA")
            colaccB = bigpool.tile([P, NPTS], BF16, tag="colaccB")
            colaccT = bigpool.tile([P, NBLK, P], BF16, tag="colaccT")
            ctree = bigpool.tile([P, NBLK, P // 2], BF16, tag="ctree")
            cmaxA = spool.tile([P, NBLK], BF16, tag="cmaxA")
            cmaxB = spool.tile([P, NBLK], BF16, tag="cmaxB")
            rowsums = spool.tile([P, NRS], FP32, tag="rowsums")

            def _ctree_reduce(dst_small, b0=0, b1=NBLK):
                """Halving max-tree over colaccT's last axis for blocks
                [b0:b1]; result copied to dst_small[:, b0:b1]."""
                w = P // 2
                nc.vector.tensor_tensor(
                    ctree[:, b0:b1, 0:w], colaccT[:, b0:b1, 0:w],
                    colaccT[:, b0:b1, w:P], op=ALU.max,
                )
                while w > 1:
                    h2 = w // 2
                    nc.vector.tensor_tensor(
                        ctree[:, b0:b1, 0:h2], ctree[:, b0:b1, 0:h2],
                        ctree[:, b0:b1, h2:w], op=ALU.max,
                    )
                    w = h2
                nc.vector.tensor_copy(dst_small[:, b0:b1], ctree[:, b0:b1, 0])

            with tc.tile_pool(name="psum", bufs=2, space="PSUM") as psum_pool:
              for rep in range(nrep):
                for c in range(CHUNKS):
                    acc_direct = colaccA if c == 0 else (
                        colaccB if c == CHUNKS // 2 else None)
                    if acc_direct is not None and stage not in ("pe",):
                        # first chunk of an accumulator: ACT drains straight
                        # into it -- no separate DVE max needed
                        e_c = acc_direct
                    else:
                        e_c = epool.tile([P, NPTS], BF16, tag="E")
                    csl = bass.ts(c, P)
                    for g in range(NGRP):
                        pt = psum_pool.tile([P, GROUP * JT], FP32, tag="D")
                        # main matmuls of the group share one lhsT load;
                        # the K=2 bias folds share another.
                        for t in range(GROUP):
                            j0 = (g * GROUP + t) * JT
                            nc.tensor.matmul(
                                pt[:, bass.ts(t, JT)],
                                lhsT=sb_xT2[:, :, csl],
                                rhs=sb_yT[:, :, j0 : j0 + JT],
                                start=True,
                                stop=False,
                                perf_mode=DR,
                            )
                        for t in range(GROUP):
                            j0 = (g * GROUP + t) * JT
                            nc.tensor.matmul(
                                pt[:, bass.ts(t, JT)],
                                lhsT=sb_flhs[:, :, csl],
                                rhs=sb_frhs[:, :, j0 : j0 + JT],
                                start=False,
                                stop=True,
                                perf_mode=DR,
                            )
                        gsl = bass.ts(g, GROUP * JT)
                        if stage == "pe":
                            # keep a consumer so matmuls aren't dead: tiny
                            # copy of one column per group
                            nc.scalar.activation(
                                e_c[:, g : g + 1], pt[:, 0:1], AF.Copy
                            )
                        else:
                            # E = exp((C - d_sh)/T), rowsum partial for free
                            nc.scalar.activation(
                                e_c[:, gsl], pt[:], AF.Exp,
                                bias=sb_bias[:, 0:1],
                                scale=-1.0 / TSOFT,
                                accum_out=rowsums[:, c * NGRP + g
                                                  : c * NGRP + g + 1],
                            )

                    if stage in ("pe", "pedrain"):
                        # tiny reader keeps each chunk's work live
                        nc.vector.tensor_copy(
                            cmaxA[0:1, c : c + 1], e_c[0:1, 0:1]
                        )
                        continue
                    # per-y col maxima accumulated elementwise across chunks
                    colacc = colaccA if c < CHUNKS // 2 else colaccB
                    if acc_direct is None:
                        for j0 in range(0, NPTS, COLACC_GRAIN):
                            sl = slice(j0, j0 + COLACC_GRAIN)
                            nc.vector.tensor_tensor(
                                colacc[:, sl], e_c[:, sl], colacc[:, sl],
                                op=ALU.max,
                            )
                    if c == CHUNKS // 2 - 1 and stage == "host":
                        # finalize A while B's chunks still run
                        nc.sync.dma_start_transpose(colaccT[:], colaccA[:])
                        _ctree_reduce(cmaxA)

                if stage == "host":
                    # finalize B (transpose in two halves so the max-tree on
                    # half 0 overlaps the DMA of half 1), then output
                    half = NPTS // 2
                    hb = NBLK // 2
                    nc.sync.dma_start_transpose(
                        colaccT[:, 0:hb, :], colaccB[:, 0:half])
                    nc.sync.dma_start_transpose(
                        colaccT[:, hb:NBLK, :], colaccB[:, half:NPTS])
                    _ctree_reduce(cmaxB, 0, hb)
                    _ctree_reduce(cmaxB, hb, NBLK)
                    cmax = spool.tile([P, NBLK], FP32, tag="cmax")
                    nc.vector.tensor_tensor(
                        cmax[:], cmaxA[:], cmaxB[:], op=ALU.max)
                    nc.sync.dma_start(parts[:, 0:NBLK], cmax[:])
                    nc.sync.dma_start(parts[:, NBLK:], rowsums[:])
                elif stage == "main":
                    lres0 = spool.tile([1, 2], FP32, tag="lres0")
                    nc.vector.tensor_copy(lres0[:, 0:1], rowsums[0:1, 0:1])
                    nc.vector.tensor_copy(lres0[:, 1:2], colaccB[0:1, 0:1])
                    nc.sync.dma_start(parts[0:1, 0:2], lres0[:])
                else:
                    lres0 = spool.tile([1, 1], FP32, tag="lres0")
                    nc.vector.tensor_copy(lres0[:], cmaxA[0:1, 0:1])
                    nc.sync.dma_start(parts[0:1, 0:1], lres0[:])

    nc.compile()
    return nc


_NC_CACHE: dict = {}


def _get_module(stage: str = "host"):
    if stage not in _NC_CACHE:
        _NC_CACHE[stage] = _build_module(stage)
    return _NC_CACHE[stage]


_RUNNER_CACHE: dict = {}


def _get_runner(stage: str = "host", donate: bool = True):
    """Build (once) a jitted SPMD callable over the 8 cores."""
    key = (stage, donate)
    if key in _RUNNER_CACHE:
        return _RUNNER_CACHE[key]

    import jax
    from jax.sharding import Mesh, PartitionSpec
    from jax.experimental.shard_map import shard_map
    import concourse.mybir as _mybir
    from concourse import bass2jax

    nc = _get_module(stage)
    bass2jax.install_neuronx_cc_hook()

    partition_name = (
        nc.partition_id_tensor.name if nc.partition_id_tensor else None
    )
    in_names: list[str] = []
    out_names: list[str] = []
    out_avals: list[jax.core.ShapedArray] = []
    zero_outs: list[np.ndarray] = []
    for alloc in nc.m.functions[0].allocations:
        if not isinstance(alloc, _mybir.MemoryLocationSet):
            continue
        name = alloc.memorylocations[0].name
        if alloc.kind == "ExternalInput":
            if name != partition_name:
                in_names.append(name)
        elif alloc.kind == "ExternalOutput":
            out_names.append(name)
            shape = tuple(alloc.tensor_shape)
            dtype = _mybir.dt.np(alloc.dtype)
            out_avals.append(jax.core.ShapedArray(shape, dtype))
            zero_outs.append(np.zeros(shape, dtype))
    n_params = len(in_names)
    n_outs = len(out_avals)
    all_names = in_names + out_names
    if partition_name is not None:
        all_names = all_names + [partition_name]

    def _body(*args):
        operands = list(args)
        if partition_name is not None:
            operands.append(bass2jax.partition_id_tensor())
        outs = bass2jax._bass_exec_p.bind(
            *operands,
            out_avals=tuple(out_avals),
            in_names=tuple(all_names),
            out_names=tuple(out_names),
            lowering_input_output_aliases=(),
            sim_require_finite=True,
            sim_require_nnan=True,
            nc=nc,
        )
        return tuple(outs)

    devices = jax.devices()[:NCORES]
    mesh = Mesh(np.asarray(devices), ("core",))
    in_specs = (PartitionSpec("core"),) * (n_params + n_outs)
    out_specs = (PartitionSpec("core"),) * n_outs
    jit_kw = (
        dict(donate_argnums=tuple(range(n_params, n_params + n_outs)))
        if donate
        else {}
    )
    sharded = jax.jit(
        shard_map(_body, mesh=mesh, in_specs=in_specs, out_specs=out_specs,
                  check_rep=False),
        keep_unused=True,
        **jit_kw,
    )
    _RUNNER_CACHE[key] = (sharded, in_names, out_names, out_avals, zero_outs,
                          mesh)
    return _RUNNER_CACHE[key]


def _run(in_maps, stage="host"):
    sharded, in_names, out_names, out_avals, zero_outs, _ = _get_runner(stage)
    concat_in = [
        np.concatenate([np.asarray(in_maps[c][n]) for c in range(NCORES)], axis=0)
        for n in in_names
    ]
    concat_zeros = [
        np.zeros((NCORES * z.shape[0], *z.shape[1:]), z.dtype) for z in zero_outs
    ]
    out_arrs = sharded(*concat_in, *concat_zeros)
    return [
        {
            n: np.asarray(out_arrs[i]).reshape(NCORES, *out_avals[i].shape)[c]
            for i, n in enumerate(out_names)
        }
        for c in range(NCORES)
    ]


def _prep_inputs(x: np.ndarray, y: np.ndarray):
    return _prep_inputs_s(x, y)[0]


def _prep_inputs_s(x: np.ndarray, y: np.ndarray):
    x = np.asarray(x, np.float32)
    y = np.asarray(y, np.float32)
    x2 = np.sum(x.astype(np.float64) ** 2, axis=1)
    y2 = np.sum(y.astype(np.float64) ** 2, axis=1)
    s = float(x2.min() + y2.min())
    x2s = (x2 - x2.min()).astype(np.float32)
    y2s = (y2 - y2.min()).astype(np.float32)

    import ml_dtypes

    f8 = np.dtype(ml_dtypes.float8_e4m3)

    def _f8split3(v):
        """v (fp32) -> three fp8 planes summing to ~v (err <= ~0.03)."""
        hi = v.astype(f8).astype(np.float32)
        mid = (v - hi).astype(f8).astype(np.float32)
        lo = (v - hi - mid).astype(f8)
        return hi.astype(f8), mid.astype(f8), lo

    # DoubleRow interleave: [features, n] -> [64, 2, n]
    yT = np.ascontiguousarray(
        y.T.reshape(2, P // 2, NPTS).transpose(1, 0, 2)
    ).astype(f8)
    # fold K=6 as [3, 2, n]: k = k_o*3 + k_i -> rows
    # [y2hi, y2mid, y2lo, 1, 1, 1] on rhs, [1, 1, 1, x2hi, x2mid, x2lo] on lhs
    y2hi, y2mid, y2lo = _f8split3(y2s)
    fold_rhs = np.empty((3, 2, NPTS), f8)
    fold_rhs[0, 0] = y2hi
    fold_rhs[1, 0] = y2mid
    fold_rhs[2, 0] = y2lo
    fold_rhs[:, 1] = np.float32(1.0).astype(f8)

    # softmin reference point: median row min (shifted space) over a small
    # row sample; exact value only affects dynamic range, not correctness
    samp = x[:: NPTS // 16][:16]
    samp2 = x2s[:: NPTS // 16][:16]
    d_samp = samp2[:, None] + y2s[None, :] - 2.0 * (samp @ y.T)
    c_sh = float(np.median(d_samp.min(axis=1)))
    actbias = np.full((P, 1), c_sh / TSOFT, np.float32)

    in_maps = []
    for c in range(NCORES):
        sl = slice(c * LOCAL, (c + 1) * LOCAL)
        xT2 = np.ascontiguousarray(
            (-2.0 * x[sl]).T.reshape(2, P // 2, LOCAL).transpose(1, 0, 2)
        ).astype(f8)
        x2hi, x2mid, x2lo = _f8split3(x2s[sl])
        fold_lhsT = np.empty((3, 2, LOCAL), f8)
        fold_lhsT[:, 0] = np.float32(1.0).astype(f8)
        fold_lhsT[0, 1] = x2hi
        fold_lhsT[1, 1] = x2mid
        fold_lhsT[2, 1] = x2lo
        in_maps.append(
            {
                "xT2": xT2,
                "yT": yT,
                "fold_lhsT": fold_lhsT,
                "fold_rhs": fold_rhs,
                "actbias": actbias,
            }
        )
    return in_maps, s, c_sh


def kernel(x: np.ndarray, y: np.ndarray, **_ignored):
    x = np.asarray(x, np.float32)
    y = np.asarray(y, np.float32)
    in_maps, s, c_sh = _prep_inputs_s(x, y)
    results = _run(in_maps, stage="host")
    parts = np.stack([results[c]["parts"] for c in range(NCORES)])  # [8,128,96]
    colmax = parts[:, :, 0:NBLK].astype(np.float64).max(axis=0)  # [128, 64]
    rowsum = parts[:, :, NBLK:].astype(np.float64)
    rowsum = rowsum.reshape(NCORES, P, CHUNKS, NGRP).sum(axis=3)  # [8,128,8]
    colmin = s + c_sh - TSOFT * np.log(colmax)
    rowmin = s + c_sh - TSOFT * np.log(rowsum)
    loss = (
        np.maximum(colmin, 0.0).sum() + np.maximum(rowmin, 0.0).sum()
    ) / NPTS
    return np.float32(loss)


# revision 19
# speedup vs baseline: 2.9851x; 1.7222x over previous
"""Chamfer loss kernel for Trainium2 (8 NeuronCores, Bass/Tile).

Problem: x, y of shape [8192, 128] fp32.
  dist[i,j] = max(||x_i||^2 + ||y_j||^2 - 2 x_i.y_j, 0)
  loss = (sum_j min_i dist[i,j] + sum_i min_j dist[i,j]) / 8192

Sharding: x rows are split across the 8 cores (1024 rows each); every core
holds all of y. Each core computes its [1024, 8192] distance tile via PE
matmuls (K = 128 features on partitions):

  PSUM tile = (-2 x_chunk^T)^T @ y^T            (K=128 fp16 matmul)
            + [ones; x2_chunk]^T @ [y2; ones]   (K=2 rank-2 bias fold)
  => PSUM[i, j] = d_sh = dist - S  (shifted squared distance)

Exp-space reductions (softmin rows / exact monotone cols):
  ScalarE drains PSUM with  E = exp((C - d_sh)/T)  (bf16), and its free
  accum_out gives per-partition rowsum partials  sum_j E  -- the entire
  per-x row reduction costs nothing beyond the drain. VectorE only
  accumulates per-y column maxima  colacc = max(colacc, E)  elementwise
  across the 8 x-chunks (exp is monotone, so max E <-> min dist exactly).
  A DMA transpose + bf16 max-tree reduces colacc across partitions.

Each core outputs parts[128, 64+32] = [col max-reduced | rowsum partials];
the host does the tiny final math:  min_dist = S + C - T*log(.)  plus the
cross-core max/sum, clamp and mean. Softmin bias at T=1.25 is ~7e-4 rel.
"""

import os
import sys

import numpy as np

sys.path.insert(0, "/opt/trn_rl_repo")
os.environ.setdefault("MYCRO_LOCAL_CACHE", "1")

import concourse.bass as bass
import concourse.bacc as bacc
import concourse.mybir as mybir
import concourse.tile as tile

FP16 = mybir.dt.float16
BF16 = mybir.dt.bfloat16
FP32 = mybir.dt.float32
FP8 = mybir.dt.float8e4
AF = mybir.ActivationFunctionType
ALU = mybir.AluOpType
DR = mybir.MatmulPerfMode.DoubleRow

NPTS = 8192          # points in x and in y
DIM = 128            # feature dim = matmul contraction K
NCORES = 8
LOCAL = NPTS // NCORES   # 1024 x-rows per core
P = 128              # partitions
CHUNKS = LOCAL // P      # 8 chunks of 128 x-rows per core
JT = 512             # j-tile width (one PSUM bank of fp32)
GROUP = 4            # j-tiles per PSUM pool buffer / ACT drain
NGRP = NPTS // (JT * GROUP)  # 4 drain groups per chunk
NBLK = NPTS // P     # 64 column blocks of 128 y-points
NRS = CHUNKS * NGRP  # 32 rowsum partial slots

TSOFT = 1.25         # softmin temperature (baked into the NEFF)

EBUFS = int(os.environ.get("K_EBUFS", "3"))
COLACC_GRAIN = int(os.environ.get("K_COLACC_GRAIN", "2048"))


def _build_module(stage: str = "host"):
    """stage: 'host' (per-core partials, host combine) | 'main' (chunk loop
    only) | 'pe' (matmuls only) | 'pedrain' (matmuls + exp drain).
    A digit suffix repeats the body (e.g. 'host17')."""
    nrep = 1
    for base in ("host", "pedrain", "pe", "main"):
        if stage.startswith(base):
            suf = stage[len(base):]
            if suf.isdigit():
                nrep = int(suf)
            stage = base
            break
    nc = bacc.Bacc(
        "TRN2",
        target_bir_lowering=False,
        debug=False,
        num_devices=NCORES,
    )

    # DoubleRow fp8 layout [Ki=67, 2, n]: k_i<64 carry the 128 features
    # (feature f -> (f % 64, f // 64)); k_i in {64,65,66} carry the rank-6
    # bias fold ([1,1,1] x [y2hi,y2mid,y2lo] and [x2hi,x2mid,x2lo] x [1,1,1])
    # so one matmul per tile produces the full shifted distance.
    KI = P // 2 + 3
    xT2 = nc.dram_tensor("xT2", [KI, 2, LOCAL], FP8, kind="ExternalInput")
    yT = nc.dram_tensor("yT", [KI, 2, NPTS], FP8, kind="ExternalInput")
    actbias = nc.dram_tensor("actbias", [P, 1], FP32, kind="ExternalInput")
    # cols 0..63: per-y col max of E (cross-partition reduced); cols 64..95:
    # rowsum partials per (chunk, group)
    parts = nc.dram_tensor("parts", [P, NBLK + NRS], FP32, kind="ExternalOutput")

    with tile.TileContext(nc) as tc:
        with (
            tc.tile_pool(name="const", bufs=1) as cpool,
            tc.tile_pool(name="big", bufs=1) as bigpool,
            tc.tile_pool(name="epool", bufs=EBUFS) as epool,
            tc.tile_pool(name="scratch", bufs=1) as spool,
        ):
            sb_xT2 = cpool.tile([KI, 2, LOCAL], FP8, tag="xT2")
            sb_yT = cpool.tile([KI, 2, NPTS], FP8, tag="yT")
            sb_bias = cpool.tile([P, 1], FP32, tag="abias")

            nc.sync.dma_start(sb_yT[:], yT[:])
            nc.sync.dma_start(sb_xT2[:], xT2[:])
            nc.sync.dma_start(sb_bias[:], actbias[:])

            # two column-max accumulators: A over chunks 0-3, B over 4-7.
            # A finalizes (transpose + partition max-tree) while B's chunks
            # still run, hiding half the tail.
            colaccA = bigpool.tile([P, NPTS], BF16, tag="colaccA")
            colaccB = bigpool.tile([P, NPTS], BF16, tag="colaccB")
            colaccT = bigpool.tile([P, NBLK, P], BF16, tag="colaccT")
            ctree = bigpool.tile([P, NBLK, P // 2], BF16, tag="ctree")
            cmaxA = spool.tile([P, NBLK], BF16, tag="cmaxA")
            cmaxB = spool.tile([P, NBLK], BF16, tag="cmaxB")
            rowsums = spool.tile([P, NRS], FP32, tag="rowsums")

            def _ctree_reduce(dst_small, b0=0, b1=NBLK):
                """Halving max-tree over colaccT's last axis for blocks
                [b0:b1]; result copied to dst_small[:, b0:b1]."""
                w = P // 2
                nc.vector.tensor_tensor(
                    ctree[:, b0:b1, 0:w], colaccT[:, b0:b1, 0:w],
                    colaccT[:, b0:b1, w:P], op=ALU.max,
                )
                while w > 1:
                    h2 = w // 2
                    nc.vector.tensor_tensor(
                        ctree[:, b0:b1, 0:h2], ctree[:, b0:b1, 0:h2],
                        ctree[:, b0:b1, h2:w], op=ALU.max,
                    )
                    w = h2
                nc.vector.tensor_copy(dst_small[:, b0:b1], ctree[:, b0:b1, 0])

            with tc.tile_pool(name="psum", bufs=2, space="PSUM") as psum_pool:
              for rep in range(nrep):
                for c in range(CHUNKS):
                    acc_direct = colaccA if c == 0 else (
                        colaccB if c == CHUNKS // 2 else None)
                    if acc_direct is not None and stage not in ("pe",):
                        # first chunk of an accumulator: ACT drains straight
                        # into it -- no separate DVE max needed
                        e_c = acc_direct
                    else:
                        e_c = epool.tile([P, NPTS], BF16, tag="E")
                    csl = bass.ts(c, P)
                    for g in range(NGRP):
                        pt = psum_pool.tile([P, GROUP * JT], FP32, tag="D")
                        # main matmuls of the group share one lhsT load;
                        # the K=2 bias folds share another.
                        for t in range(GROUP):
                            j0 = (g * GROUP + t) * JT
                            nc.tensor.matmul(
                                pt[:, bass.ts(t, JT)],
                                lhsT=sb_xT2[:, :, csl],
                                rhs=sb_yT[:, :, j0 : j0 + JT],
                                start=True,
                                stop=True,
                                perf_mode=DR,
                            )
                        gsl = bass.ts(g, GROUP * JT)
                        if stage == "pe":
                            # keep a consumer so matmuls aren't dead: tiny
                            # copy of one column per group
                            nc.scalar.activation(
                                e_c[:, g : g + 1], pt[:, 0:1], AF.Copy
                            )
                        else:
                            # E = exp((C - d_sh)/T), rowsum partial for free
                            nc.scalar.activation(
                                e_c[:, gsl], pt[:], AF.Exp,
                                bias=sb_bias[:, 0:1],
                                scale=-1.0 / TSOFT,
                                accum_out=rowsums[:, c * NGRP + g
                                                  : c * NGRP + g + 1],
                            )

                    if stage in ("pe", "pedrain"):
                        # tiny reader keeps each chunk's work live
                        nc.vector.tensor_copy(
                            cmaxA[0:1, c : c + 1], e_c[0:1, 0:1]
                        )
                        continue
                    # per-y col maxima accumulated elementwise across chunks
                    colacc = colaccA if c < CHUNKS // 2 else colaccB
                    if acc_direct is None:
                        for j0 in range(0, NPTS, COLACC_GRAIN):
                            sl = slice(j0, j0 + COLACC_GRAIN)
                            nc.vector.tensor_tensor(
                                colacc[:, sl], e_c[:, sl], colacc[:, sl],
                                op=ALU.max,
                            )
                    if c == CHUNKS // 2 - 1 and stage == "host":
                        # finalize A while B's chunks still run
                        nc.sync.dma_start_transpose(colaccT[:], colaccA[:])
                        _ctree_reduce(cmaxA)

                if stage == "host":
                    # finalize B (transpose in two halves so the max-tree on
                    # half 0 overlaps the DMA of half 1), then output
                    half = NPTS // 2
                    hb = NBLK // 2
                    nc.sync.dma_start_transpose(
                        colaccT[:, 0:hb, :], colaccB[:, 0:half])
                    nc.sync.dma_start_transpose(
                        colaccT[:, hb:NBLK, :], colaccB[:, half:NPTS])
                    _ctree_reduce(cmaxB, 0, hb)
                    _ctree_reduce(cmaxB, hb, NBLK)
                    cmax = spool.tile([P, NBLK], FP32, tag="cmax")
                    nc.vector.tensor_tensor(
                        cmax[:], cmaxA[:], cmaxB[:], op=ALU.max)
                    nc.sync.dma_start(parts[:, 0:NBLK], cmax[:])
                    nc.sync.dma_start(parts[:, NBLK:], rowsums[:])
                elif stage == "main":
                    lres0 = spool.tile([1, 2], FP32, tag="lres0")
                    nc.vector.tensor_copy(lres0[:, 0:1], rowsums[0:1, 0:1])
                    nc.vector.tensor_copy(lres0[:, 1:2], colaccB[0:1, 0:1])
                    nc.sync.dma_start(parts[0:1, 0:2], lres0[:])
                else:
                    lres0 = spool.tile([1, 1], FP32, tag="lres0")
                    nc.vector.tensor_copy(lres0[:], cmaxA[0:1, 0:1])
                    nc.sync.dma_start(parts[0:1, 0:1], lres0[:])

    nc.compile()
    return nc


_NC_CACHE: dict = {}


def _get_module(stage: str = "host"):
    if stage not in _NC_CACHE:
        _NC_CACHE[stage] = _build_module(stage)
    return _NC_CACHE[stage]


_RUNNER_CACHE: dict = {}


def _get_runner(stage: str = "host", donate: bool = True):
    """Build (once) a jitted SPMD callable over the 8 cores."""
    key = (stage, donate)
    if key in _RUNNER_CACHE:
        return _RUNNER_CACHE[key]

    import jax
    from jax.sharding import Mesh, PartitionSpec
    from jax.experimental.shard_map import shard_map
    import concourse.mybir as _mybir
    from concourse import bass2jax

    nc = _get_module(stage)
    bass2jax.install_neuronx_cc_hook()

    partition_name = (
        nc.partition_id_tensor.name if nc.partition_id_tensor else None
    )
    in_names: list[str] = []
    out_names: list[str] = []
    out_avals: list[jax.core.ShapedArray] = []
    zero_outs: list[np.ndarray] = []
    for alloc in nc.m.functions[0].allocations:
        if not isinstance(alloc, _mybir.MemoryLocationSet):
            continue
        name = alloc.memorylocations[0].name
        if alloc.kind == "ExternalInput":
            if name != partition_name:
                in_names.append(name)
        elif alloc.kind == "ExternalOutput":
            out_names.append(name)
            shape = tuple(alloc.tensor_shape)
            dtype = _mybir.dt.np(alloc.dtype)
            out_avals.append(jax.core.ShapedArray(shape, dtype))
            zero_outs.append(np.zeros(shape, dtype))
    n_params = len(in_names)
    n_outs = len(out_avals)
    all_names = in_names + out_names
    if partition_name is not None:
        all_names = all_names + [partition_name]

    def _body(*args):
        operands = list(args)
        if partition_name is not None:
            operands.append(bass2jax.partition_id_tensor())
        outs = bass2jax._bass_exec_p.bind(
            *operands,
            out_avals=tuple(out_avals),
            in_names=tuple(all_names),
            out_names=tuple(out_names),
            lowering_input_output_aliases=(),
            sim_require_finite=True,
            sim_require_nnan=True,
            nc=nc,
        )
        return tuple(outs)

    devices = jax.devices()[:NCORES]
    mesh = Mesh(np.asarray(devices), ("core",))
    in_specs = (PartitionSpec("core"),) * (n_params + n_outs)
    out_specs = (PartitionSpec("core"),) * n_outs
    jit_kw = (
        dict(donate_argnums=tuple(range(n_params, n_params + n_outs)))
        if donate
        else {}
    )
    sharded = jax.jit(
        shard_map(_body, mesh=mesh, in_specs=in_specs, out_specs=out_specs,
                  check_rep=False),
        keep_unused=True,
        **jit_kw,
    )
    _RUNNER_CACHE[key] = (sharded, in_names, out_names, out_avals, zero_outs,
                          mesh)
    return _RUNNER_CACHE[key]


def _run(in_maps, stage="host"):
    sharded, in_names, out_names, out_avals, zero_outs, _ = _get_runner(stage)
    concat_in = [
        np.concatenate([np.asarray(in_maps[c][n]) for c in range(NCORES)], axis=0)
        for n in in_names
    ]
    concat_zeros = [
        np.zeros((NCORES * z.shape[0], *z.shape[1:]), z.dtype) for z in zero_outs
    ]
    out_arrs = sharded(*concat_in, *concat_zeros)
    return [
        {
            n: np.asarray(out_arrs[i]).reshape(NCORES, *out_avals[i].shape)[c]
            for i, n in enumerate(out_names)
        }
        for c in range(NCORES)
    ]


def _prep_inputs(x: np.ndarray, y: np.ndarray):
    return _prep_inputs_s(x, y)[0]


def _prep_inputs_s(x: np.ndarray, y: np.ndarray):
    x = np.asarray(x, np.float32)
    y = np.asarray(y, np.float32)
    x2 = np.sum(x.astype(np.float64) ** 2, axis=1)
    y2 = np.sum(y.astype(np.float64) ** 2, axis=1)
    s = float(x2.min() + y2.min())
    x2s = (x2 - x2.min()).astype(np.float32)
    y2s = (y2 - y2.min()).astype(np.float32)

    import ml_dtypes

    f8 = np.dtype(ml_dtypes.float8_e4m3)

    def _f8split3(v):
        """v (fp32) -> three fp8 planes summing to ~v (err <= ~0.03)."""
        hi = v.astype(f8).astype(np.float32)
        mid = (v - hi).astype(f8).astype(np.float32)
        lo = (v - hi - mid).astype(f8)
        return hi.astype(f8), mid.astype(f8), lo

    # DoubleRow interleave [Ki=67, 2, n]: features then the 3 fold pairs
    KI = P // 2 + 3
    yT = np.empty((KI, 2, NPTS), f8)
    yT[: P // 2] = y.T.reshape(2, P // 2, NPTS).transpose(1, 0, 2).astype(f8)
    y2hi, y2mid, y2lo = _f8split3(y2s)
    yT[P // 2, 0] = y2hi
    yT[P // 2 + 1, 0] = y2mid
    yT[P // 2 + 2, 0] = y2lo
    yT[P // 2 :, 1] = np.float32(1.0).astype(f8)

    # softmin reference point: median row min (shifted space) over a small
    # row sample; exact value only affects dynamic range, not correctness
    samp = x[:: NPTS // 16][:16]
    samp2 = x2s[:: NPTS // 16][:16]
    d_samp = samp2[:, None] + y2s[None, :] - 2.0 * (samp @ y.T)
    c_sh = float(np.median(d_samp.min(axis=1)))
    actbias = np.full((P, 1), c_sh / TSOFT, np.float32)

    in_maps = []
    for c in range(NCORES):
        sl = slice(c * LOCAL, (c + 1) * LOCAL)
        xT2 = np.empty((KI, 2, LOCAL), f8)
        xT2[: P // 2] = (
            (-2.0 * x[sl]).T.reshape(2, P // 2, LOCAL).transpose(1, 0, 2)
        ).astype(f8)
        x2hi, x2mid, x2lo = _f8split3(x2s[sl])
        xT2[P // 2 :, 0] = np.float32(1.0).astype(f8)
        xT2[P // 2, 1] = x2hi
        xT2[P // 2 + 1, 1] = x2mid
        xT2[P // 2 + 2, 1] = x2lo
        in_maps.append(
            {
                "xT2": xT2,
                "yT": yT,
                "actbias": actbias,
            }
        )
    return in_maps, s, c_sh


def kernel(x: np.ndarray, y: np.ndarray, **_ignored):
    x = np.asarray(x, np.float32)
    y = np.asarray(y, np.float32)
    in_maps, s, c_sh = _prep_inputs_s(x, y)
    results = _run(in_maps, stage="host")
    parts = np.stack([results[c]["parts"] for c in range(NCORES)])  # [8,128,96]
    colmax = parts[:, :, 0:NBLK].astype(np.float64).max(axis=0)  # [128, 64]
    rowsum = parts[:, :, NBLK:].astype(np.float64)
    rowsum = rowsum.reshape(NCORES, P, CHUNKS, NGRP).sum(axis=3)  # [8,128,8]
    colmin = s + c_sh - TSOFT * np.log(colmax)
    rowmin = s + c_sh - TSOFT * np.log(rowsum)
    loss = (
        np.maximum(colmin, 0.0).sum() + np.maximum(rowmin, 0.0).sum()
    ) / NPTS
    return np.float32(loss)


# revision 29
# speedup vs baseline: 3.2131x; 1.0764x over previous
"""Chamfer loss kernel for Trainium2 (8 NeuronCores, Bass/Tile).

Problem: x, y of shape [8192, 128] fp32.
  dist[i,j] = max(||x_i||^2 + ||y_j||^2 - 2 x_i.y_j, 0)
  loss = (sum_j min_i dist[i,j] + sum_i min_j dist[i,j]) / 8192

Sharding: x rows are split across the 8 cores (1024 rows each); every core
holds all of y. Each core computes its [1024, 8192] distance tile via PE
matmuls (K = 128 features on partitions):

  PSUM tile = (-2 x_chunk^T)^T @ y^T            (K=128 fp16 matmul)
            + [ones; x2_chunk]^T @ [y2; ones]   (K=2 rank-2 bias fold)
  => PSUM[i, j] = d_sh = dist - S  (shifted squared distance)

Exp-space reductions (softmin rows / exact monotone cols):
  ScalarE drains PSUM with  E = exp((C - d_sh)/T)  (bf16), and its free
  accum_out gives per-partition rowsum partials  sum_j E  -- the entire
  per-x row reduction costs nothing beyond the drain. VectorE only
  accumulates per-y column maxima  colacc = max(colacc, E)  elementwise
  across the 8 x-chunks (exp is monotone, so max E <-> min dist exactly).
  A DMA transpose + bf16 max-tree reduces colacc across partitions.

Each core outputs parts[128, 64+32] = [col max-reduced | rowsum partials];
the host does the tiny final math:  min_dist = S + C - T*log(.)  plus the
cross-core max/sum, clamp and mean. Softmin bias at T=1.25 is ~7e-4 rel.
"""

import os
import sys

import numpy as np

sys.path.insert(0, "/opt/trn_rl_repo")
os.environ.setdefault("MYCRO_LOCAL_CACHE", "1")

import concourse.bass as bass
import concourse.bacc as bacc
import concourse.mybir as mybir
import concourse.tile as tile

FP16 = mybir.dt.float16
BF16 = mybir.dt.bfloat16
FP32 = mybir.dt.float32
FP8 = mybir.dt.float8e4
AF = mybir.ActivationFunctionType
ALU = mybir.AluOpType
DR = mybir.MatmulPerfMode.DoubleRow

NPTS = 8192          # points in x and in y
DIM = 128            # feature dim = matmul contraction K
NCORES = 8
LOCAL = NPTS // NCORES   # 1024 x-rows per core
P = 128              # partitions
CHUNKS = LOCAL // P      # 8 chunks of 128 x-rows per core
JT = 512             # j-tile width (one PSUM bank of fp32)
GROUP = 4            # j-tiles per PSUM pool buffer / ACT drain
NGRP = NPTS // (JT * GROUP)  # 4 drain groups per chunk
NBLK = NPTS // P     # 64 column blocks of 128 y-points
NRS = CHUNKS * NGRP  # 32 rowsum partial slots
HW = 1024            # trailing columns per chunk handled by the DVE hard path
SOFTW = NPTS - HW    # 7168 columns on the ACT exp path
HBLK = HW // P       # 8 hard column blocks
SBLK = SOFTW // P    # 56 soft column blocks

TSOFT = 1.25         # softmin temperature (baked into the NEFF)

EBUFS = int(os.environ.get("K_EBUFS", "3"))
COLACC_GRAIN = int(os.environ.get("K_COLACC_GRAIN", "2048"))


def _build_module(stage: str = "host"):
    """stage: 'host' (per-core partials, host combine) | 'main' (chunk loop
    only) | 'pe' (matmuls only) | 'pedrain' (matmuls + exp drain).
    A digit suffix repeats the body (e.g. 'host17')."""
    nrep = 1
    for base in ("host", "pedrain", "pe", "main"):
        if stage.startswith(base):
            suf = stage[len(base):]
            if suf.isdigit():
                nrep = int(suf)
            stage = base
            break
    nc = bacc.Bacc(
        "TRN2",
        target_bir_lowering=False,
        debug=False,
        num_devices=NCORES,
    )

    # DoubleRow fp8 layout [Ki=67, 2, n]: k_i<64 carry the 128 features
    # (feature f -> (f % 64, f // 64)); k_i in {64,65,66} carry the rank-6
    # bias fold ([1,1,1] x [y2hi,y2mid,y2lo] and [x2hi,x2mid,x2lo] x [1,1,1])
    # so one matmul per tile produces the full shifted distance.
    KI = P // 2 + 3
    xT2 = nc.dram_tensor("xT2", [KI, 2, LOCAL], FP8, kind="ExternalInput")
    yT = nc.dram_tensor("yT", [KI, 2, NPTS], FP8, kind="ExternalInput")
    actbias = nc.dram_tensor("actbias", [P, 1], FP32, kind="ExternalInput")
    # cols 0..55: per-y col max of E (soft path, cross-partition reduced);
    # 56..63: per-y col min of d_sh (hard path); 64..95: rowsum partials per
    # (chunk, group); 96..103: hard row-min partials per chunk
    parts = nc.dram_tensor(
        "parts", [P, NBLK + NRS + CHUNKS], FP32, kind="ExternalOutput")

    with tile.TileContext(nc) as tc:
        with (
            tc.tile_pool(name="const", bufs=1) as cpool,
            tc.tile_pool(name="big", bufs=1) as bigpool,
            tc.tile_pool(name="epool", bufs=EBUFS) as epool,
            tc.tile_pool(name="scratch", bufs=1) as spool,
        ):
            sb_xT2 = cpool.tile([KI, 2, LOCAL], FP8, tag="xT2")
            sb_yT = cpool.tile([KI, 2, NPTS], FP8, tag="yT")
            sb_bias = cpool.tile([P, 1], FP32, tag="abias")

            nc.sync.dma_start(sb_yT[:], yT[:])
            nc.sync.dma_start(sb_xT2[:], xT2[:])
            nc.sync.dma_start(sb_bias[:], actbias[:])

            # two column-max accumulators: A over chunks 0-3, B over 4-7.
            # A finalizes (transpose + partition max-tree) while B's chunks
            # still run, hiding half the tail.
            colaccA = bigpool.tile([P, NPTS], BF16, tag="colaccA")
            colaccB = bigpool.tile([P, NPTS], BF16, tag="colaccB")
            colaccT = bigpool.tile([P, NBLK, P], BF16, tag="colaccT")
            ctree = bigpool.tile([P, NBLK, P // 2], BF16, tag="ctree")
            cmaxA = spool.tile([P, SBLK], BF16, tag="cmaxA")
            cmaxB = spool.tile([P, SBLK], BF16, tag="cmaxB")
            rowsums = spool.tile([P, NRS], FP32, tag="rowsums")
            colaccH = bigpool.tile([P, HW], BF16, tag="colaccH")
            rminH = spool.tile([P, CHUNKS], FP32, tag="rminH")
            cminH = spool.tile([P, HBLK], BF16, tag="cminH")

            def _ctree_reduce(dst_small, b0=0, b1=NBLK, d0=None, op=ALU.max):
                """Halving reduce-tree over colaccT's last axis for blocks
                [b0:b1]; result copied to dst_small[:, d0:d0+(b1-b0)]."""
                if d0 is None:
                    d0 = b0
                w = P // 2
                nc.vector.tensor_tensor(
                    ctree[:, b0:b1, 0:w], colaccT[:, b0:b1, 0:w],
                    colaccT[:, b0:b1, w:P], op=op,
                )
                while w > 1:
                    h2 = w // 2
                    nc.vector.tensor_tensor(
                        ctree[:, b0:b1, 0:h2], ctree[:, b0:b1, 0:h2],
                        ctree[:, b0:b1, h2:w], op=op,
                    )
                    w = h2
                nc.vector.tensor_copy(
                    dst_small[:, d0 : d0 + b1 - b0], ctree[:, b0:b1, 0])

            with tc.tile_pool(name="psum", bufs=2, space="PSUM") as psum_pool:
              for rep in range(nrep):
                for c in range(CHUNKS):
                    acc_direct = colaccA if c == 0 else (
                        colaccB if c == CHUNKS // 2 else None)
                    if acc_direct is not None and stage not in ("pe",):
                        # first chunk of an accumulator: ACT drains straight
                        # into it -- no separate DVE max needed
                        e_c = acc_direct
                    else:
                        e_c = epool.tile([P, NPTS], BF16, tag="E")
                    csl = bass.ts(c, P)
                    for g in range(NGRP):
                        pt = psum_pool.tile([P, GROUP * JT], FP32, tag="D")
                        # main matmuls of the group share one lhsT load;
                        # the K=2 bias folds share another.
                        for t in range(GROUP):
                            j0 = (g * GROUP + t) * JT
                            nc.tensor.matmul(
                                pt[:, bass.ts(t, JT)],
                                lhsT=sb_xT2[:, :, csl],
                                rhs=sb_yT[:, :, j0 : j0 + JT],
                                start=True,
                                stop=True,
                                perf_mode=DR,
                            )
                        if stage == "pe":
                            # keep a consumer so matmuls aren't dead: tiny
                            # copy of one column per group
                            nc.scalar.activation(
                                e_c[:, g : g + 1], pt[:, 0:1], AF.Copy
                            )
                            continue
                        # E = exp((C - d_sh)/T), rowsum partial for free.
                        # The last HW columns of the chunk go to the DVE
                        # hard path straight from PSUM instead.
                        g0 = g * GROUP * JT
                        aw = GROUP * JT if g < NGRP - 1 else GROUP * JT - HW
                        nc.scalar.activation(
                            e_c[:, g0 : g0 + aw], pt[:, 0:aw], AF.Exp,
                            bias=sb_bias[:, 0:1],
                            scale=-1.0 / TSOFT,
                            accum_out=rowsums[:, c * NGRP + g
                                              : c * NGRP + g + 1],
                        )
                        if g == NGRP - 1 and stage != "pedrain":
                            hsl = slice(GROUP * JT - HW, GROUP * JT)
                            if c == 0:
                                nc.vector.tensor_copy(colaccH[:], pt[:, hsl])
                            else:
                                nc.vector.tensor_tensor(
                                    colaccH[:], pt[:, hsl], colaccH[:],
                                    op=ALU.min,
                                )
                            nc.vector.tensor_reduce(
                                rminH[:, c : c + 1], pt[:, hsl],
                                axis=mybir.AxisListType.X, op=ALU.min,
                            )

                    if stage in ("pe", "pedrain"):
                        # tiny reader keeps each chunk's work live
                        nc.vector.tensor_copy(
                            cmaxA[0:1, c : c + 1], e_c[0:1, 0:1]
                        )
                        continue
                    # per-y col maxima accumulated elementwise across chunks
                    colacc = colaccA if c < CHUNKS // 2 else colaccB
                    if acc_direct is None:
                        for j0 in range(0, SOFTW, COLACC_GRAIN):
                            sl = slice(j0, min(j0 + COLACC_GRAIN, SOFTW))
                            nc.vector.tensor_tensor(
                                colacc[:, sl], e_c[:, sl], colacc[:, sl],
                                op=ALU.max,
                            )
                    if c == CHUNKS // 2 - 1 and stage == "host":
                        # finalize A while B's chunks still run
                        nc.sync.dma_start_transpose(
                            colaccT[:, 0:SBLK, :], colaccA[:, 0:SOFTW])
                        _ctree_reduce(cmaxA, 0, SBLK)

                if stage == "host":
                    # finalize B (transpose in two halves so the max-tree on
                    # half 0 overlaps the DMA of half 1), plus the hard-path
                    # accumulator, then output
                    half = SOFTW // 2
                    hb = SBLK // 2
                    nc.sync.dma_start_transpose(
                        colaccT[:, 0:hb, :], colaccB[:, 0:half])
                    nc.sync.dma_start_transpose(
                        colaccT[:, hb:SBLK, :], colaccB[:, half:SOFTW])
                    nc.sync.dma_start_transpose(
                        colaccT[:, SBLK:NBLK, :], colaccH[:])
                    _ctree_reduce(cmaxB, 0, hb)
                    _ctree_reduce(cmaxB, hb, SBLK)
                    _ctree_reduce(cminH, SBLK, NBLK, d0=0, op=ALU.min)
                    cmax = spool.tile([P, NBLK], FP32, tag="cmax")
                    nc.vector.tensor_tensor(
                        cmax[:, 0:SBLK], cmaxA[:], cmaxB[:], op=ALU.max)
                    nc.vector.tensor_copy(cmax[:, SBLK:NBLK], cminH[:])
                    nc.sync.dma_start(parts[:, 0:NBLK], cmax[:])
                    nc.sync.dma_start(
                        parts[:, NBLK : NBLK + NRS], rowsums[:])
                    nc.sync.dma_start(parts[:, NBLK + NRS :], rminH[:])
                elif stage == "main":
                    lres0 = spool.tile([1, 2], FP32, tag="lres0")
                    nc.vector.tensor_copy(lres0[:, 0:1], rowsums[0:1, 0:1])
                    nc.vector.tensor_copy(lres0[:, 1:2], colaccB[0:1, 0:1])
                    nc.sync.dma_start(parts[0:1, 0:2], lres0[:])
                else:
                    lres0 = spool.tile([1, 1], FP32, tag="lres0")
                    nc.vector.tensor_copy(lres0[:], cmaxA[0:1, 0:1])
                    nc.sync.dma_start(parts[0:1, 0:1], lres0[:])

    nc.compile()
    return nc


_NC_CACHE: dict = {}


def _get_module(stage: str = "host"):
    if stage not in _NC_CACHE:
        _NC_CACHE[stage] = _build_module(stage)
    return _NC_CACHE[stage]


_RUNNER_CACHE: dict = {}


def _get_runner(stage: str = "host", donate: bool = True):
    """Build (once) a jitted SPMD callable over the 8 cores."""
    key = (stage, donate)
    if key in _RUNNER_CACHE:
        return _RUNNER_CACHE[key]

    import jax
    from jax.sharding import Mesh, PartitionSpec
    from jax.experimental.shard_map import shard_map
    import concourse.mybir as _mybir
    from concourse import bass2jax

    nc = _get_module(stage)
    bass2jax.install_neuronx_cc_hook()

    partition_name = (
        nc.partition_id_tensor.name if nc.partition_id_tensor else None
    )
    in_names: list[str] = []
    out_names: list[str] = []
    out_avals: list[jax.core.ShapedArray] = []
    zero_outs: list[np.ndarray] = []
    for alloc in nc.m.functions[0].allocations:
        if not isinstance(alloc, _mybir.MemoryLocationSet):
            continue
        name = alloc.memorylocations[0].name
        if alloc.kind == "ExternalInput":
            if name != partition_name:
                in_names.append(name)
        elif alloc.kind == "ExternalOutput":
            out_names.append(name)
            shape = tuple(alloc.tensor_shape)
            dtype = _mybir.dt.np(alloc.dtype)
            out_avals.append(jax.core.ShapedArray(shape, dtype))
            zero_outs.append(np.zeros(shape, dtype))
    n_params = len(in_names)
    n_outs = len(out_avals)
    all_names = in_names + out_names
    if partition_name is not None:
        all_names = all_names + [partition_name]

    def _body(*args):
        operands = list(args)
        if partition_name is not None:
            operands.append(bass2jax.partition_id_tensor())
        outs = bass2jax._bass_exec_p.bind(
            *operands,
            out_avals=tuple(out_avals),
            in_names=tuple(all_names),
            out_names=tuple(out_names),
            lowering_input_output_aliases=(),
            sim_require_finite=True,
            sim_require_nnan=True,
            nc=nc,
        )
        return tuple(outs)

    devices = jax.devices()[:NCORES]
    mesh = Mesh(np.asarray(devices), ("core",))
    in_specs = (PartitionSpec("core"),) * (n_params + n_outs)
    out_specs = (PartitionSpec("core"),) * n_outs
    jit_kw = (
        dict(donate_argnums=tuple(range(n_params, n_params + n_outs)))
        if donate
        else {}
    )
    sharded = jax.jit(
        shard_map(_body, mesh=mesh, in_specs=in_specs, out_specs=out_specs,
                  check_rep=False),
        keep_unused=True,
        **jit_kw,
    )
    _RUNNER_CACHE[key] = (sharded, in_names, out_names, out_avals, zero_outs,
                          mesh)
    return _RUNNER_CACHE[key]


def _run(in_maps, stage="host"):
    sharded, in_names, out_names, out_avals, zero_outs, _ = _get_runner(stage)
    concat_in = [
        np.concatenate([np.asarray(in_maps[c][n]) for c in range(NCORES)], axis=0)
        for n in in_names
    ]
    concat_zeros = [
        np.zeros((NCORES * z.shape[0], *z.shape[1:]), z.dtype) for z in zero_outs
    ]
    out_arrs = sharded(*concat_in, *concat_zeros)
    return [
        {
            n: np.asarray(out_arrs[i]).reshape(NCORES, *out_avals[i].shape)[c]
            for i, n in enumerate(out_names)
        }
        for c in range(NCORES)
    ]


def _prep_inputs(x: np.ndarray, y: np.ndarray):
    return _prep_inputs_s(x, y)[0]


def _prep_inputs_s(x: np.ndarray, y: np.ndarray):
    x = np.asarray(x, np.float32)
    y = np.asarray(y, np.float32)
    x2 = np.sum(x.astype(np.float64) ** 2, axis=1)
    y2 = np.sum(y.astype(np.float64) ** 2, axis=1)
    s = float(x2.min() + y2.min())
    x2s = (x2 - x2.min()).astype(np.float32)
    y2s = (y2 - y2.min()).astype(np.float32)

    import ml_dtypes

    f8 = np.dtype(ml_dtypes.float8_e4m3)

    def _f8split3(v):
        """v (fp32) -> three fp8 planes summing to ~v (err <= ~0.03)."""
        hi = v.astype(f8).astype(np.float32)
        mid = (v - hi).astype(f8).astype(np.float32)
        lo = (v - hi - mid).astype(f8)
        return hi.astype(f8), mid.astype(f8), lo

    # DoubleRow interleave [Ki=67, 2, n]: features then the 3 fold pairs
    KI = P // 2 + 3
    yT = np.empty((KI, 2, NPTS), f8)
    yT[: P // 2] = y.T.reshape(2, P // 2, NPTS).transpose(1, 0, 2).astype(f8)
    y2hi, y2mid, y2lo = _f8split3(y2s)
    yT[P // 2, 0] = y2hi
    yT[P // 2 + 1, 0] = y2mid
    yT[P // 2 + 2, 0] = y2lo
    yT[P // 2 :, 1] = np.float32(1.0).astype(f8)

    # softmin reference point: median row min (shifted space) over a small
    # row sample; exact value only affects dynamic range, not correctness
    samp = x[:: NPTS // 16][:16]
    samp2 = x2s[:: NPTS // 16][:16]
    d_samp = samp2[:, None] + y2s[None, :] - 2.0 * (samp @ y.T)
    c_sh = float(np.median(d_samp.min(axis=1)))
    actbias = np.full((P, 1), c_sh / TSOFT, np.float32)

    in_maps = []
    for c in range(NCORES):
        sl = slice(c * LOCAL, (c + 1) * LOCAL)
        xT2 = np.empty((KI, 2, LOCAL), f8)
        xT2[: P // 2] = (
            (-2.0 * x[sl]).T.reshape(2, P // 2, LOCAL).transpose(1, 0, 2)
        ).astype(f8)
        x2hi, x2mid, x2lo = _f8split3(x2s[sl])
        xT2[P // 2 :, 0] = np.float32(1.0).astype(f8)
        xT2[P // 2, 1] = x2hi
        xT2[P // 2 + 1, 1] = x2mid
        xT2[P // 2 + 2, 1] = x2lo
        in_maps.append(
            {
                "xT2": xT2,
                "yT": yT,
                "actbias": actbias,
            }
        )
    return in_maps, s, c_sh


def kernel(x: np.ndarray, y: np.ndarray, **_ignored):
    x = np.asarray(x, np.float32)
    y = np.asarray(y, np.float32)
    in_maps, s, c_sh = _prep_inputs_s(x, y)
    results = _run(in_maps, stage="host")
    parts = np.stack([results[c]["parts"] for c in range(NCORES)])  # [8,128,104]
    colmax = parts[:, :, 0:SBLK].astype(np.float64).max(axis=0)  # soft cols
    colminH = parts[:, :, SBLK:NBLK].astype(np.float64).min(axis=0) + s
    rowsum = parts[:, :, NBLK : NBLK + NRS].astype(np.float64)
    rowsum = rowsum.reshape(NCORES, P, CHUNKS, NGRP).sum(axis=3)  # [8,128,8]
    rminh = parts[:, :, NBLK + NRS :].astype(np.float64) + s  # [8,128,8]
    colmin = s + c_sh - TSOFT * np.log(np.maximum(colmax, 1e-300))
    rowmin = s + c_sh - TSOFT * np.log(np.maximum(rowsum, 1e-300))
    rowmin = np.minimum(rowmin, rminh)
    loss = (
        np.maximum(colmin, 0.0).sum() + np.maximum(colminH, 0.0).sum()
        + np.maximum(rowmin, 0.0).sum()
    ) / NPTS
    return np.float32(loss)


# revision 30
# speedup vs baseline: 4.8846x; 1.5202x over previous
"""Chamfer loss kernel for Trainium2 (8 NeuronCores, Bass/Tile).

Problem: x, y of shape [8192, 128] fp32.
  dist[i,j] = max(||x_i||^2 + ||y_j||^2 - 2 x_i.y_j, 0)
  loss = (sum_j min_i dist[i,j] + sum_i min_j dist[i,j]) / 8192

Sharding: x rows are split across the 8 cores (1024 rows each); every core
holds all of y. Each core computes its [1024, 8192] distance tile via a
single fp8e4m3 DoubleRow matmul per 512-wide j-tile: the [Ki=67, 2, n]
operands pack the 128 features (0.5 cyc/col) PLUS three split-precision
fold pairs ([1,1,1] x [y2hi,y2mid,y2lo] and [x2hi,x2mid,x2lo] x [1,1,1],
each plane fp8, summed error <= ~0.03), so

  PSUM[i, j] = d_sh = dist - S  (full shifted squared distance)

with no separate bias-fold matmuls. Exp-space reductions:
  ScalarE drains PSUM with  E = exp((C - d_sh)/T)  (bf16, T=1.25), and its
  free accum_out gives per-partition rowsum partials  sum_j E  -- the whole
  per-x row reduction (softmin) costs nothing beyond the drain. VectorE
  accumulates per-y column maxima  colacc = max(colacc, E)  across the 8
  x-chunks (exp is monotone, so max E <-> min dist exactly), then a DMA
  transpose + bf16 max-tree reduces colacc across partitions. The trailing
  HW=1024 columns of each chunk skip the exp path entirely: VectorE takes
  them straight from PSUM (tensor_tensor min colacc + tensor_reduce row
  min), rebalancing ScalarE vs VectorE load.

Each core outputs parts[128, 56+8+32+8] = [soft col-max | hard col-min |
rowsum partials | hard row-min partials]; the host does the tiny final
math:  min_dist = S + C - T*log(.)  plus cross-core max/min/sum, clamp and
mean. End-to-end rel err ~5e-5.
"""

import os
import sys

import numpy as np

sys.path.insert(0, "/opt/trn_rl_repo")
os.environ.setdefault("MYCRO_LOCAL_CACHE", "1")

import concourse.bass as bass
import concourse.bacc as bacc
import concourse.mybir as mybir
import concourse.tile as tile

FP16 = mybir.dt.float16
BF16 = mybir.dt.bfloat16
FP32 = mybir.dt.float32
FP8 = mybir.dt.float8e4
AF = mybir.ActivationFunctionType
ALU = mybir.AluOpType
DR = mybir.MatmulPerfMode.DoubleRow

NPTS = 8192          # points in x and in y
DIM = 128            # feature dim = matmul contraction K
NCORES = 8
LOCAL = NPTS // NCORES   # 1024 x-rows per core
P = 128              # partitions
CHUNKS = LOCAL // P      # 8 chunks of 128 x-rows per core
JT = 512             # j-tile width (one PSUM bank of fp32)
GROUP = 4            # j-tiles per PSUM pool buffer / ACT drain
NGRP = NPTS // (JT * GROUP)  # 4 drain groups per chunk
NBLK = NPTS // P     # 64 column blocks of 128 y-points
NRS = CHUNKS * NGRP  # 32 rowsum partial slots
HW = 1024            # trailing columns per chunk handled by the DVE hard path
SOFTW = NPTS - HW    # 7168 columns on the ACT exp path
HBLK = HW // P       # 8 hard column blocks
SBLK = SOFTW // P    # 56 soft column blocks

TSOFT = 1.25         # softmin temperature (baked into the NEFF)

EBUFS = int(os.environ.get("K_EBUFS", "3"))
COLACC_GRAIN = int(os.environ.get("K_COLACC_GRAIN", "2048"))


def _build_module(stage: str = "host"):
    """stage: 'host' (per-core partials, host combine) | 'main' (chunk loop
    only) | 'pe' (matmuls only) | 'pedrain' (matmuls + exp drain).
    A digit suffix repeats the body (e.g. 'host17')."""
    nrep = 1
    for base in ("host", "pedrain", "pe", "main"):
        if stage.startswith(base):
            suf = stage[len(base):]
            if suf.isdigit():
                nrep = int(suf)
            stage = base
            break
    nc = bacc.Bacc(
        "TRN2",
        target_bir_lowering=False,
        debug=False,
        num_devices=NCORES,
    )

    # DoubleRow fp8 layout [Ki=67, 2, n]: k_i<64 carry the 128 features
    # (feature f -> (f % 64, f // 64)); k_i in {64,65,66} carry the rank-6
    # bias fold ([1,1,1] x [y2hi,y2mid,y2lo] and [x2hi,x2mid,x2lo] x [1,1,1])
    # so one matmul per tile produces the full shifted distance.
    KI = P // 2 + 3
    xT2 = nc.dram_tensor("xT2", [KI, 2, LOCAL], FP8, kind="ExternalInput")
    yT = nc.dram_tensor("yT", [KI, 2, NPTS], FP8, kind="ExternalInput")
    actbias = nc.dram_tensor("actbias", [P, 1], FP32, kind="ExternalInput")
    # cols 0..55: per-y col max of E (soft path, cross-partition reduced);
    # 56..63: per-y col min of d_sh (hard path); 64..95: rowsum partials per
    # (chunk, group); 96..103: hard row-min partials per chunk
    parts = nc.dram_tensor(
        "parts", [P, NBLK + NRS + CHUNKS], FP32, kind="ExternalOutput")

    with tile.TileContext(nc) as tc:
        with (
            tc.tile_pool(name="const", bufs=1) as cpool,
            tc.tile_pool(name="big", bufs=1) as bigpool,
            tc.tile_pool(name="epool", bufs=EBUFS) as epool,
            tc.tile_pool(name="scratch", bufs=1) as spool,
        ):
            sb_xT2 = cpool.tile([KI, 2, LOCAL], FP8, tag="xT2")
            sb_yT = cpool.tile([KI, 2, NPTS], FP8, tag="yT")
            sb_bias = cpool.tile([P, 1], FP32, tag="abias")

            nc.sync.dma_start(sb_yT[:], yT[:])
            nc.sync.dma_start(sb_xT2[:], xT2[:])
            nc.sync.dma_start(sb_bias[:], actbias[:])

            # two column-max accumulators: A over chunks 0-3, B over 4-7.
            # A finalizes (transpose + partition max-tree) while B's chunks
            # still run, hiding half the tail.
            colaccA = bigpool.tile([P, NPTS], BF16, tag="colaccA")
            colaccB = bigpool.tile([P, NPTS], BF16, tag="colaccB")
            colaccT = bigpool.tile([P, NBLK, P], BF16, tag="colaccT")
            ctree = bigpool.tile([P, NBLK, P // 2], BF16, tag="ctree")
            cmaxA = spool.tile([P, SBLK], BF16, tag="cmaxA")
            cmaxB = spool.tile([P, SBLK], BF16, tag="cmaxB")
            rowsums = spool.tile([P, NRS], FP32, tag="rowsums")
            colaccH = bigpool.tile([P, HW], BF16, tag="colaccH")
            rminH = spool.tile([P, CHUNKS], FP32, tag="rminH")
            cminH = spool.tile([P, HBLK], BF16, tag="cminH")

            def _ctree_reduce(dst_small, b0=0, b1=NBLK, d0=None, op=ALU.max):
                """Halving reduce-tree over colaccT's last axis for blocks
                [b0:b1]; result copied to dst_small[:, d0:d0+(b1-b0)]."""
                if d0 is None:
                    d0 = b0
                w = P // 2
                nc.vector.tensor_tensor(
                    ctree[:, b0:b1, 0:w], colaccT[:, b0:b1, 0:w],
                    colaccT[:, b0:b1, w:P], op=op,
                )
                while w > 1:
                    h2 = w // 2
                    nc.vector.tensor_tensor(
                        ctree[:, b0:b1, 0:h2], ctree[:, b0:b1, 0:h2],
                        ctree[:, b0:b1, h2:w], op=op,
                    )
                    w = h2
                nc.vector.tensor_copy(
                    dst_small[:, d0 : d0 + b1 - b0], ctree[:, b0:b1, 0])

            with tc.tile_pool(name="psum", bufs=2, space="PSUM") as psum_pool:
              for rep in range(nrep):
                for c in range(CHUNKS):
                    acc_direct = colaccA if c == 0 else (
                        colaccB if c == CHUNKS // 2 else None)
                    if acc_direct is not None and stage not in ("pe",):
                        # first chunk of an accumulator: ACT drains straight
                        # into it -- no separate DVE max needed
                        e_c = acc_direct
                    else:
                        e_c = epool.tile([P, NPTS], BF16, tag="E")
                    csl = bass.ts(c, P)
                    for g in range(NGRP):
                        pt = psum_pool.tile([P, GROUP * JT], FP32, tag="D")
                        # main matmuls of the group share one lhsT load;
                        # the K=2 bias folds share another.
                        for t in range(GROUP):
                            j0 = (g * GROUP + t) * JT
                            nc.tensor.matmul(
                                pt[:, bass.ts(t, JT)],
                                lhsT=sb_xT2[:, :, csl],
                                rhs=sb_yT[:, :, j0 : j0 + JT],
                                start=True,
                                stop=True,
                                perf_mode=DR,
                            )
                        if stage == "pe":
                            # keep a consumer so matmuls aren't dead: tiny
                            # copy of one column per group
                            nc.scalar.activation(
                                e_c[:, g : g + 1], pt[:, 0:1], AF.Copy
                            )
                            continue
                        # E = exp((C - d_sh)/T), rowsum partial for free.
                        # The last HW columns of the chunk go to the DVE
                        # hard path straight from PSUM instead.
                        g0 = g * GROUP * JT
                        aw = GROUP * JT if g < NGRP - 1 else GROUP * JT - HW
                        nc.scalar.activation(
                            e_c[:, g0 : g0 + aw], pt[:, 0:aw], AF.Exp,
                            bias=sb_bias[:, 0:1],
                            scale=-1.0 / TSOFT,
                            accum_out=rowsums[:, c * NGRP + g
                                              : c * NGRP + g + 1],
                        )
                        if g == NGRP - 1 and stage != "pedrain":
                            hsl = slice(GROUP * JT - HW, GROUP * JT)
                            if c == 0:
                                nc.vector.tensor_copy(colaccH[:], pt[:, hsl])
                            else:
                                nc.vector.tensor_tensor(
                                    colaccH[:], pt[:, hsl], colaccH[:],
                                    op=ALU.min,
                                )
                            nc.vector.tensor_reduce(
                                rminH[:, c : c + 1], pt[:, hsl],
                                axis=mybir.AxisListType.X, op=ALU.min,
                            )

                    if stage in ("pe", "pedrain"):
                        # tiny reader keeps each chunk's work live
                        nc.vector.tensor_copy(
                            cmaxA[0:1, c : c + 1], e_c[0:1, 0:1]
                        )
                        continue
                    # per-y col maxima accumulated elementwise across chunks
                    colacc = colaccA if c < CHUNKS // 2 else colaccB
                    if acc_direct is None:
                        for j0 in range(0, SOFTW, COLACC_GRAIN):
                            sl = slice(j0, min(j0 + COLACC_GRAIN, SOFTW))
                            nc.vector.tensor_tensor(
                                colacc[:, sl], e_c[:, sl], colacc[:, sl],
                                op=ALU.max,
                            )
                    if c == CHUNKS // 2 - 1 and stage == "host":
                        # finalize A while B's chunks still run
                        nc.sync.dma_start_transpose(
                            colaccT[:, 0:SBLK, :], colaccA[:, 0:SOFTW])
                        _ctree_reduce(cmaxA, 0, SBLK)

                if stage == "host":
                    # finalize B (transpose in two halves so the max-tree on
                    # half 0 overlaps the DMA of half 1), plus the hard-path
                    # accumulator, then output
                    half = SOFTW // 2
                    hb = SBLK // 2
                    nc.sync.dma_start_transpose(
                        colaccT[:, 0:hb, :], colaccB[:, 0:half])
                    nc.sync.dma_start_transpose(
                        colaccT[:, hb:SBLK, :], colaccB[:, half:SOFTW])
                    nc.sync.dma_start_transpose(
                        colaccT[:, SBLK:NBLK, :], colaccH[:])
                    _ctree_reduce(cmaxB, 0, hb)
                    _ctree_reduce(cmaxB, hb, SBLK)
                    _ctree_reduce(cminH, SBLK, NBLK, d0=0, op=ALU.min)
                    cmax = spool.tile([P, NBLK], FP32, tag="cmax")
                    nc.vector.tensor_tensor(
                        cmax[:, 0:SBLK], cmaxA[:], cmaxB[:], op=ALU.max)
                    nc.vector.tensor_copy(cmax[:, SBLK:NBLK], cminH[:])
                    nc.sync.dma_start(parts[:, 0:NBLK], cmax[:])
                    nc.sync.dma_start(
                        parts[:, NBLK : NBLK + NRS], rowsums[:])
                    nc.sync.dma_start(parts[:, NBLK + NRS :], rminH[:])
                elif stage == "main":
                    lres0 = spool.tile([1, 2], FP32, tag="lres0")
                    nc.vector.tensor_copy(lres0[:, 0:1], rowsums[0:1, 0:1])
                    nc.vector.tensor_copy(lres0[:, 1:2], colaccB[0:1, 0:1])
                    nc.sync.dma_start(parts[0:1, 0:2], lres0[:])
                else:
                    lres0 = spool.tile([1, 1], FP32, tag="lres0")
                    nc.vector.tensor_copy(lres0[:], cmaxA[0:1, 0:1])
                    nc.sync.dma_start(parts[0:1, 0:1], lres0[:])

    nc.compile()
    return nc


_NC_CACHE: dict = {}


def _get_module(stage: str = "host"):
    if stage not in _NC_CACHE:
        _NC_CACHE[stage] = _build_module(stage)
    return _NC_CACHE[stage]


_RUNNER_CACHE: dict = {}


def _get_runner(stage: str = "host", donate: bool = True):
    """Build (once) a jitted SPMD callable over the 8 cores."""
    key = (stage, donate)
    if key in _RUNNER_CACHE:
        return _RUNNER_CACHE[key]

    import jax
    from jax.sharding import Mesh, PartitionSpec
    from jax.experimental.shard_map import shard_map
    import concourse.mybir as _mybir
    from concourse import bass2jax

    nc = _get_module(stage)
    bass2jax.install_neuronx_cc_hook()

    partition_name = (
        nc.partition_id_tensor.name if nc.partition_id_tensor else None
    )
    in_names: list[str] = []
    out_names: list[str] = []
    out_avals: list[jax.core.ShapedArray] = []
    zero_outs: list[np.ndarray] = []
    for alloc in nc.m.functions[0].allocations:
        if not isinstance(alloc, _mybir.MemoryLocationSet):
            continue
        name = alloc.memorylocations[0].name
        if alloc.kind == "ExternalInput":
            if name != partition_name:
                in_names.append(name)
        elif alloc.kind == "ExternalOutput":
            out_names.append(name)
            shape = tuple(alloc.tensor_shape)
            dtype = _mybir.dt.np(alloc.dtype)
            out_avals.append(jax.core.ShapedArray(shape, dtype))
            zero_outs.append(np.zeros(shape, dtype))
    n_params = len(in_names)
    n_outs = len(out_avals)
    all_names = in_names + out_names
    if partition_name is not None:
        all_names = all_names + [partition_name]

    def _body(*args):
        operands = list(args)
        if partition_name is not None:
            operands.append(bass2jax.partition_id_tensor())
        outs = bass2jax._bass_exec_p.bind(
            *operands,
            out_avals=tuple(out_avals),
            in_names=tuple(all_names),
            out_names=tuple(out_names),
            lowering_input_output_aliases=(),
            sim_require_finite=True,
            sim_require_nnan=True,
            nc=nc,
        )
        return tuple(outs)

    devices = jax.devices()[:NCORES]
    mesh = Mesh(np.asarray(devices), ("core",))
    in_specs = (PartitionSpec("core"),) * (n_params + n_outs)
    out_specs = (PartitionSpec("core"),) * n_outs
    jit_kw = (
        dict(donate_argnums=tuple(range(n_params, n_params + n_outs)))
        if donate
        else {}
    )
    sharded = jax.jit(
        shard_map(_body, mesh=mesh, in_specs=in_specs, out_specs=out_specs,
                  check_rep=False),
        keep_unused=True,
        **jit_kw,
    )
    _RUNNER_CACHE[key] = (sharded, in_names, out_names, out_avals, zero_outs,
                          mesh)
    return _RUNNER_CACHE[key]


def _run(in_maps, stage="host"):
    sharded, in_names, out_names, out_avals, zero_outs, _ = _get_runner(stage)
    concat_in = [
        np.concatenate([np.asarray(in_maps[c][n]) for c in range(NCORES)], axis=0)
        for n in in_names
    ]
    concat_zeros = [
        np.zeros((NCORES * z.shape[0], *z.shape[1:]), z.dtype) for z in zero_outs
    ]
    out_arrs = sharded(*concat_in, *concat_zeros)
    return [
        {
            n: np.asarray(out_arrs[i]).reshape(NCORES, *out_avals[i].shape)[c]
            for i, n in enumerate(out_names)
        }
        for c in range(NCORES)
    ]


def _prep_inputs(x: np.ndarray, y: np.ndarray):
    return _prep_inputs_s(x, y)[0]


def _prep_inputs_s(x: np.ndarray, y: np.ndarray):
    x = np.asarray(x, np.float32)
    y = np.asarray(y, np.float32)
    x2 = np.sum(x.astype(np.float64) ** 2, axis=1)
    y2 = np.sum(y.astype(np.float64) ** 2, axis=1)
    s = float(x2.min() + y2.min())
    x2s = (x2 - x2.min()).astype(np.float32)
    y2s = (y2 - y2.min()).astype(np.float32)

    import ml_dtypes

    f8 = np.dtype(ml_dtypes.float8_e4m3)

    def _f8split3(v):
        """v (fp32) -> three fp8 planes summing to ~v (err <= ~0.03)."""
        hi = v.astype(f8).astype(np.float32)
        mid = (v - hi).astype(f8).astype(np.float32)
        lo = (v - hi - mid).astype(f8)
        return hi.astype(f8), mid.astype(f8), lo

    # DoubleRow interleave [Ki=67, 2, n]: features then the 3 fold pairs
    KI = P // 2 + 3
    yT = np.empty((KI, 2, NPTS), f8)
    yT[: P // 2] = y.T.reshape(2, P // 2, NPTS).transpose(1, 0, 2).astype(f8)
    y2hi, y2mid, y2lo = _f8split3(y2s)
    yT[P // 2, 0] = y2hi
    yT[P // 2 + 1, 0] = y2mid
    yT[P // 2 + 2, 0] = y2lo
    yT[P // 2 :, 1] = np.float32(1.0).astype(f8)

    # softmin reference point: median row min (shifted space) over a small
    # row sample; exact value only affects dynamic range, not correctness
    samp = x[:: NPTS // 16][:16]
    samp2 = x2s[:: NPTS // 16][:16]
    d_samp = samp2[:, None] + y2s[None, :] - 2.0 * (samp @ y.T)
    c_sh = float(np.median(d_samp.min(axis=1)))
    actbias = np.full((P, 1), c_sh / TSOFT, np.float32)

    in_maps = []
    for c in range(NCORES):
        sl = slice(c * LOCAL, (c + 1) * LOCAL)
        xT2 = np.empty((KI, 2, LOCAL), f8)
        xT2[: P // 2] = (
            (-2.0 * x[sl]).T.reshape(2, P // 2, LOCAL).transpose(1, 0, 2)
        ).astype(f8)
        x2hi, x2mid, x2lo = _f8split3(x2s[sl])
        xT2[P // 2 :, 0] = np.float32(1.0).astype(f8)
        xT2[P // 2, 1] = x2hi
        xT2[P // 2 + 1, 1] = x2mid
        xT2[P // 2 + 2, 1] = x2lo
        in_maps.append(
            {
                "xT2": xT2,
                "yT": yT,
                "actbias": actbias,
            }
        )
    return in_maps, s, c_sh


def kernel(x: np.ndarray, y: np.ndarray, **_ignored):
    x = np.asarray(x, np.float32)
    y = np.asarray(y, np.float32)
    in_maps, s, c_sh = _prep_inputs_s(x, y)
    results = _run(in_maps, stage="host")
    parts = np.stack([results[c]["parts"] for c in range(NCORES)])  # [8,128,104]
    colmax = parts[:, :, 0:SBLK].astype(np.float64).max(axis=0)  # soft cols
    colminH = parts[:, :, SBLK:NBLK].astype(np.float64).min(axis=0) + s
    rowsum = parts[:, :, NBLK : NBLK + NRS].astype(np.float64)
    rowsum = rowsum.reshape(NCORES, P, CHUNKS, NGRP).sum(axis=3)  # [8,128,8]
    rminh = parts[:, :, NBLK + NRS :].astype(np.float64) + s  # [8,128,8]
    colmin = s + c_sh - TSOFT * np.log(np.maximum(colmax, 1e-300))
    rowmin = s + c_sh - TSOFT * np.log(np.maximum(rowsum, 1e-300))
    rowmin = np.minimum(rowmin, rminh)
    loss = (
        np.maximum(colmin, 0.0).sum() + np.maximum(colminH, 0.0).sum()
        + np.maximum(rowmin, 0.0).sum()
    ) / NPTS
    return np.float32(loss)
